# revision 43
# baseline (speedup 1.0000x reference)
"""Additive (Bahdanau) attention TRN2 kernel.

reference:
    q = query @ W_q.T                         # [N,A]
    k = einsum('nlh,ah->nla', keys, W_k)      # [N,L,A]
    energy = tanh(k + q[:,None,:])
    scores = einsum('nla,a->nl', energy, v)   # [N,L]
    attn = softmax(scores, -1)                # mask is all-True: no-op
    context = einsum('nl,nlh->nh', attn, keys)
    return (context, attn)

Sharding: data-parallel over N across 8 cores (4 batches per core);
W_q/W_k/v replicated. Everything runs on one SPMD Bass program.

Per-core layout (per batch b, per l-slab of LQ=512):
  matmul1  k_projT[a, l] += W_kT[h, a].T @ keysT[h, l]   (fp32r, PSUM [128,512] x4)
  ACT      energyT = tanh(k_projT + qT[a])               (bias = per-partition q)
  matmul   scores[1, l]  += v[a].T @ energyT[a, l]       (fp32r)
  ACT      w_exp = exp(scores), accum_out -> slab sum    (no max needed:
           |scores| <= ||v||_1 * 1 ~= 16, exp safe in fp32)
  PE       transpose w_exp chunks -> attnT [l=128, 1]
  matmul   ctx[1, h] += attnT.T @ keysN[l, h]            (fp32r, streaming)
  epilogue context = ctx / sum(exp), attn = w_exp / sum(exp)

keysT (transposed) and keysN (natural) copies are prepared host-side so
every DMA is wide and contiguous; the contraction dim always sits on SBUF
partitions.
"""

import numpy as np

N, L, H, A = 32, 2048, 1024, 1024
N_CORES = 8
NBC = N // N_CORES  # batches per core


def _build(nbc=NBC, l=L, h=H, a=A, lq=512, a_grp=4):
    import concourse.mybir as mybir
    from concourse import bacc
    from concourse.tile import TileContext

    F32 = mybir.dt.float32
    F32R = mybir.dt.float32r
    TANH = mybir.ActivationFunctionType.Tanh
    EXP = mybir.ActivationFunctionType.Exp
    AXX = mybir.AxisListType.X

    hc = h // 128      # h-chunks (contraction for matmul1)
    ac = a // 128      # a-tiles
    nlq = l // lq      # l-slabs per batch
    lcq = lq // 128    # 128-chunks per l-slab
    n_ah = ac // a_grp # a-groups (psum tiles in flight)
    hw_chunk = 512 if h % 512 == 0 else h  # ctx matmul N-chunk (<=1 PSUM bank)
    n_hw = h // hw_chunk

    nc = bacc.Bacc("TRN2", target_bir_lowering=False)

    keysT_d = nc.dram_tensor("keysT", [nbc, h, l], F32R, kind="ExternalInput")
    keysN_d = nc.dram_tensor("keysN", [nbc, l, h], F32R, kind="ExternalInput")
    qryT_d = nc.dram_tensor("queryT", [h, nbc], F32R, kind="ExternalInput")
    WqT_d = nc.dram_tensor("WqT", [h, a], F32R, kind="ExternalInput")
    WkT_d = nc.dram_tensor("WkT", [h, a], F32R, kind="ExternalInput")
    v_d = nc.dram_tensor("v", [a], F32R, kind="ExternalInput")
    ctx_d = nc.dram_tensor("context", [nbc, h], F32, kind="ExternalOutput")
    attn_d = nc.dram_tensor("attn", [nbc, l], F32, kind="ExternalOutput")

    with TileContext(nc) as tc:
        with (
            tc.tile_pool(name="const", bufs=1) as cpool,
            tc.tile_pool(name="kT", bufs=2) as kTpool,
            tc.tile_pool(name="wq", bufs=1) as wqpool,
            tc.tile_pool(name="kN", bufs=2) as kNpool,
            tc.tile_pool(name="en", bufs=2) as enpool,
            tc.tile_pool(name="small", bufs=2) as spool,
            tc.tile_pool(name="outs", bufs=1) as opool,
            tc.tile_pool(name="pk", bufs=1, space="PSUM") as pkpool,
            tc.tile_pool(name="pmisc", bufs=1, space="PSUM") as pmpool,
            tc.tile_pool(name="pctx", bufs=1, space="PSUM") as pcpool,
        ):
            # -- resident constants.  WkT pieces go first on the Sync
            #    issue queue (the first matmuls need them); query/v ride
            #    the GpSimd queue in parallel. -------------------------
            from concourse.masks import make_identity

            wq_v = WqT_d.ap().rearrange("(c k) a -> k c a", k=128)
            wk = cpool.tile([128, hc, a], F32R)
            wk_v = WkT_d.ap().rearrange("(c k) a -> k c a", k=128)
            for hh in range(0, hc, 2):
                nc.sync.dma_start(out=wk[:, hh:hh + 2, :],
                                  in_=wk_v[:, hh:hh + 2, :])
            qry = cpool.tile([128, hc, nbc], F32R)
            nc.gpsimd.dma_start(out=qry[:, :, :],
                                in_=qryT_d.ap().rearrange("(c k) n -> k c n",
                                                          k=128))
            v_sb = cpool.tile([128, ac], F32R)
            nc.gpsimd.dma_start(out=v_sb[:, :],
                                in_=v_d.ap().rearrange("(c k) -> k c", k=128))
            ident = cpool.tile([32, 32], F32)
            make_identity(nc, ident[:, :])

            # -- q projection: q[n, a] = sum_h queryT[h, n] * WqT[h, a],
            #    then PE-transpose to qT[a-part, n].  Emitted lazily,
            #    interleaved with the first keys slab's matmuls so the PE
            #    starts on mm1 as soon as WkT/keysT land. ---------------
            qn = opool.tile([nbc, a], F32, tag="qn")
            qT = cpool.tile([128, ac, nbc], F32)
            at_per_chunk = hw_chunk // 128
            qp_state = {"emitted": 0}

            def emit_qproj_chunk():
                hh2 = qp_state["emitted"]
                qp_state["emitted"] += 1
                wqc = wqpool.tile([128, hc, hw_chunk], F32R, tag="wq")
                for hh in range(hc):
                    nc.sync.dma_start(
                        out=wqc[:, hh, :],
                        in_=wq_v[:, hh, hh2 * hw_chunk:(hh2 + 1) * hw_chunk])
                pq = pmpool.tile([128, hw_chunk], F32, tag="ps")
                for hh in range(hc):
                    nc.tensor.matmul(
                        pq[:nbc, :],
                        qry[:, hh, :],
                        wqc[:, hh, :],
                        start=(hh == 0),
                        stop=(hh == hc - 1),
                    )
                nc.vector.tensor_copy(
                    qn[:, hh2 * hw_chunk:(hh2 + 1) * hw_chunk], pq[:nbc, :])
                for at in range(hh2 * at_per_chunk, (hh2 + 1) * at_per_chunk):
                    pqt = pmpool.tile([128, nbc], F32, tag="pt")
                    nc.tensor.transpose(
                        pqt[:, :], qn[:, at * 128:(at + 1) * 128],
                        ident[:nbc, :nbc],
                    )
                    nc.vector.tensor_copy(qT[:, at, :], pqt[:, :])

            def ensure_qproj(up_to_at):
                while qp_state["emitted"] * at_per_chunk < up_to_at:
                    emit_qproj_chunk()

            # -- main loop ------------------------------------------------
            for b in range(nbc):
                w_exp = spool.tile([1, l], F32, tag="wexp")
                sume = spool.tile([1, nlq], F32, tag="sume")
                ctxp = pcpool.tile([1, h], F32, tag="ctx")
                for q in range(nlq):
                    # slab DMAs split into pieces: each dma_start lands on
                    # one DMA queue, so splitting multiplies transfer BW
                    kT = kTpool.tile([128, hc, lq], F32R, tag="kT")
                    kT_src = keysT_d.ap()[b].rearrange("(c k) l -> k c l", k=128)[
                        :, :, q * lq:(q + 1) * lq]
                    step = 1 if (b == 0 and q == 0) else 2
                    for hh in range(0, hc, step):
                        nc.sync.dma_start(out=kT[:, hh:hh + step, :],
                                          in_=kT_src[:, hh:hh + step, :])
                    kN = kNpool.tile([128, lcq, h], F32R, tag="kN")
                    kN_src = keysN_d.ap()[b].rearrange("(t k) h -> k t h", k=128)[
                        :, q * lcq:(q + 1) * lcq, :]
                    for t in range(0, lcq, 2):
                        nc.sync.dma_start(out=kN[:, t:t + 2, :],
                                          in_=kN_src[:, t:t + 2, :])
                    en = enpool.tile([128, ac, lq], F32R, tag="en")
                    # k-projection + tanh, a_grp PSUM tiles in flight.
                    # On the very first slab, q-proj chunks are emitted
                    # between a group's matmuls and its tanhs.
                    for g in range(n_ah):
                        pks = []
                        for ai in range(a_grp):
                            at = g * a_grp + ai
                            pk = pkpool.tile([128, lq], F32, tag=f"pk{ai}")
                            pks.append(pk)
                            for hh in range(hc):
                                nc.tensor.matmul(
                                    pk[:, :],
                                    wk[:, hh, at * 128:(at + 1) * 128],
                                    kT[:, hh, :],
                                    start=(hh == 0),
                                    stop=(hh == hc - 1),
                                )
                        if b == 0 and q == 0:
                            ensure_qproj((g + 1) * a_grp)
                        for ai in range(a_grp):
                            at = g * a_grp + ai
                            nc.scalar.activation(
                                en[:, at, :], pks[ai][:, :], TANH,
                                bias=qT[:, at, b:b + 1],
                            )
                    # scores for this slab
                    ps = pmpool.tile([1, lq], F32, tag="ps")
                    for at in range(ac):
                        nc.tensor.matmul(
                            ps[:, :],
                            v_sb[:, at:at + 1],
                            en[:, at, :],
                            start=(at == 0),
                            stop=(at == ac - 1),
                        )
                    nc.scalar.activation(
                        w_exp[:, q * lq:(q + 1) * lq], ps[:, :], EXP,
                        accum_out=sume[:, q:q + 1],
                    )
                    # transpose exp weights into partition dim
                    pt = pmpool.tile([128, lcq], F32, tag="pt")
                    for j in range(lcq):
                        nc.tensor.transpose(
                            pt[:, j:j + 1],
                            w_exp[:, q * lq + j * 128:q * lq + (j + 1) * 128],
                            ident[:1, :1],
                        )
                    aT = spool.tile([128, lcq], F32R, tag="aT")
                    nc.vector.tensor_copy(aT[:, :], pt[:, :])
                    # context accumulation (unnormalized)
                    for j in range(lcq):
                        for hh2 in range(n_hw):
                            nc.tensor.matmul(
                                ctxp[:, hh2 * hw_chunk:(hh2 + 1) * hw_chunk],
                                aT[:, j:j + 1],
                                kN[:, j, hh2 * hw_chunk:(hh2 + 1) * hw_chunk],
                                start=(q == 0 and j == 0),
                                stop=(q == nlq - 1 and j == lcq - 1),
                            )
                # batch epilogue: normalize
                tot = spool.tile([1, 1], F32, tag="tot")
                nc.vector.reduce_sum(tot[:, :], sume[:, :], axis=AXX)
                inv = spool.tile([1, 1], F32, tag="inv")
                nc.vector.reciprocal(inv[:, :], tot[:, :])
                ctx_sb = opool.tile([1, h], F32, tag="ctxsb")
                nc.vector.tensor_scalar_mul(ctx_sb[:, :], ctxp[:, :], inv[:, :])
                nc.sync.dma_start(out=ctx_d.ap()[b:b + 1, :], in_=ctx_sb[:, :])
                attn_sb = opool.tile([1, l], F32, tag="attnsb")
                nc.vector.tensor_scalar_mul(attn_sb[:, :], w_exp[:, :], inv[:, :])
                nc.sync.dma_start(out=attn_d.ap()[b:b + 1, :], in_=attn_sb[:, :])

    nc.compile()
    return nc


_NC_CACHE = {}


def _get_nc():
    if "nc" not in _NC_CACHE:
        _NC_CACHE["nc"] = _build()
    return _NC_CACHE["nc"]


def _in_maps(query, keys, W_q, W_k, v):
    query = np.asarray(query, dtype=np.float32)
    keys = np.asarray(keys, dtype=np.float32)
    W_q = np.asarray(W_q, dtype=np.float32)
    W_k = np.asarray(W_k, dtype=np.float32)
    v = np.asarray(v, dtype=np.float32)

    WqT = np.ascontiguousarray(W_q.T)
    WkT = np.ascontiguousarray(W_k.T)

    in_maps = []
    for c in range(N_CORES):
        sl = slice(c * NBC, (c + 1) * NBC)
        in_maps.append({
            "keysT": np.ascontiguousarray(keys[sl].transpose(0, 2, 1)),
            "keysN": np.ascontiguousarray(keys[sl]),
            "queryT": np.ascontiguousarray(query[sl].T),
            "WqT": WqT,
            "WkT": WkT,
            "v": v,
        })
    return in_maps


def _run(in_maps, **kw):
    from concourse.bass_utils import run_bass_kernel_spmd

    nc = _get_nc()
    res = run_bass_kernel_spmd(nc, in_maps, list(range(N_CORES)), **kw)
    context = np.concatenate([r["context"] for r in res.results], axis=0)
    attn = np.concatenate([r["attn"] for r in res.results], axis=0)
    return (context, attn), res


def kernel(query, keys, mask, W_q, W_k, v):
    out, _ = _run(_in_maps(query, keys, W_q, W_k, v))
    return out


# revision 50
# speedup vs baseline: 1.0695x; 1.0695x over previous
"""Additive (Bahdanau) attention TRN2 kernel.

reference:
    q = query @ W_q.T                         # [N,A]
    k = einsum('nlh,ah->nla', keys, W_k)      # [N,L,A]
    energy = tanh(k + q[:,None,:])
    scores = einsum('nla,a->nl', energy, v)   # [N,L]
    attn = softmax(scores, -1)                # mask is all-True: no-op
    context = einsum('nl,nlh->nh', attn, keys)
    return (context, attn)

Sharding: data-parallel over N across 8 cores (4 batches per core);
W_q/W_k/v replicated. Everything runs on one SPMD Bass program.

Per-core layout (per batch b, per l-slab of LQ=512):
  matmul1  k_projT[a, l] += W_kT[h, a].T @ keysT[h, l]   (fp32r, PSUM [128,512] x4)
  ACT      energyT = tanh(k_projT + qT[a])               (bias = per-partition q)
  matmul   scores[1, l]  += v[a].T @ energyT[a, l]       (fp32r)
  ACT      w_exp = exp(scores), accum_out -> slab sum    (no max needed:
           |scores| <= ||v||_1 * 1 ~= 16, exp safe in fp32)
  PE       transpose w_exp chunks -> attnT [l=128, 1]
  matmul   ctx[1, h] += attnT.T @ keysN[l, h]            (fp32r, streaming)
  epilogue context = ctx / sum(exp), attn = w_exp / sum(exp)

keysT (transposed) and keysN (natural) copies are prepared host-side so
every DMA is wide and contiguous; the contraction dim always sits on SBUF
partitions.
"""

import numpy as np

N, L, H, A = 32, 2048, 1024, 1024
N_CORES = 8
NBC = N // N_CORES  # batches per core


def _build(nbc=NBC, l=L, h=H, a=A, lq=512, a_grp=4):
    import concourse.mybir as mybir
    from concourse import bacc
    from concourse.tile import TileContext

    F32 = mybir.dt.float32
    F32R = mybir.dt.float32r
    TANH = mybir.ActivationFunctionType.Tanh
    EXP = mybir.ActivationFunctionType.Exp
    COPY = mybir.ActivationFunctionType.Copy
    AXX = mybir.AxisListType.X

    hc = h // 128      # h-chunks (contraction for matmul1)
    ac = a // 128      # a-tiles
    nlq = l // lq      # l-slabs per batch
    lcq = lq // 128    # 128-chunks per l-slab
    n_ah = ac // a_grp # a-groups (psum tiles in flight)
    hw_chunk = 512 if h % 512 == 0 else h  # ctx matmul N-chunk (<=1 PSUM bank)
    n_hw = h // hw_chunk

    nc = bacc.Bacc("TRN2", target_bir_lowering=False)

    keysT_d = nc.dram_tensor("keysT", [nbc, h, l], F32R, kind="ExternalInput")
    keysN_d = nc.dram_tensor("keysN", [nbc, l, h], F32R, kind="ExternalInput")
    qryT_d = nc.dram_tensor("queryT", [h, nbc], F32R, kind="ExternalInput")
    WqT_d = nc.dram_tensor("WqT", [h, a], F32R, kind="ExternalInput")
    WkT_d = nc.dram_tensor("WkT", [h, a], F32R, kind="ExternalInput")
    v_d = nc.dram_tensor("v", [a], F32, kind="ExternalInput")
    ctx_d = nc.dram_tensor("context", [nbc, h], F32, kind="ExternalOutput")
    attn_d = nc.dram_tensor("attn", [nbc, l], F32, kind="ExternalOutput")

    with TileContext(nc) as tc:
        with (
            tc.tile_pool(name="const", bufs=1) as cpool,
            tc.tile_pool(name="kT", bufs=2) as kTpool,
            tc.tile_pool(name="wq", bufs=1) as wqpool,
            tc.tile_pool(name="kN", bufs=2) as kNpool,
            tc.tile_pool(name="en", bufs=2) as enpool,
            tc.tile_pool(name="small", bufs=2) as spool,
            tc.tile_pool(name="outs", bufs=1) as opool,
            tc.tile_pool(name="pk", bufs=1, space="PSUM") as pkpool,
            tc.tile_pool(name="pmisc", bufs=1, space="PSUM") as pmpool,
            tc.tile_pool(name="pctx", bufs=1, space="PSUM") as pcpool,
        ):
            # -- resident constants.  WkT pieces go first on the Sync
            #    issue queue (the first matmuls need them); query/v ride
            #    the GpSimd queue in parallel. -------------------------
            from concourse.masks import make_identity

            wq_v = WqT_d.ap().rearrange("(c k) a -> k c a", k=128)
            wk = cpool.tile([128, hc, a], F32R)
            wk_v = WkT_d.ap().rearrange("(c k) a -> k c a", k=128)
            for hh in range(0, hc, 2):
                nc.sync.dma_start(out=wk[:, hh:hh + 2, :],
                                  in_=wk_v[:, hh:hh + 2, :])
            qry = cpool.tile([128, hc, nbc], F32R)
            nc.gpsimd.dma_start(out=qry[:, :, :],
                                in_=qryT_d.ap().rearrange("(c k) n -> k c n",
                                                          k=128))
            v_sb = cpool.tile([128, ac], F32)
            nc.gpsimd.dma_start(out=v_sb[:, :],
                                in_=v_d.ap().rearrange("(c k) -> k c", k=128))
            ident = cpool.tile([32, 32], F32)
            make_identity(nc, ident[:, :])
            ones32 = cpool.tile([128, 1], F32)
            nc.gpsimd.memset(ones32[:, :], 1.0)
            ones_r = cpool.tile([128, 1], F32R)
            nc.vector.tensor_copy(ones_r[:, :], ones32[:, :])

            # -- q projection: q[n, a] = sum_h queryT[h, n] * WqT[h, a],
            #    then PE-transpose to qT[a-part, n].  Emitted lazily,
            #    interleaved with the first keys slab's matmuls so the PE
            #    starts on mm1 as soon as WkT/keysT land. ---------------
            qn = opool.tile([nbc, a], F32, tag="qn")
            qT = cpool.tile([128, ac, nbc], F32)
            at_per_chunk = hw_chunk // 128
            qp_state = {"emitted": 0}

            def emit_qproj_chunk():
                hh2 = qp_state["emitted"]
                qp_state["emitted"] += 1
                wqc = wqpool.tile([128, hc, hw_chunk], F32R, tag="wq")
                for hh in range(hc):
                    nc.sync.dma_start(
                        out=wqc[:, hh, :],
                        in_=wq_v[:, hh, hh2 * hw_chunk:(hh2 + 1) * hw_chunk])
                pq = pmpool.tile([128, hw_chunk], F32, tag="ps")
                for hh in range(hc):
                    nc.tensor.matmul(
                        pq[:nbc, :],
                        qry[:, hh, :],
                        wqc[:, hh, :],
                        start=(hh == 0),
                        stop=(hh == hc - 1),
                    )
                nc.vector.tensor_copy(
                    qn[:, hh2 * hw_chunk:(hh2 + 1) * hw_chunk], pq[:nbc, :])
                for at in range(hh2 * at_per_chunk, (hh2 + 1) * at_per_chunk):
                    pqt = pmpool.tile([128, nbc], F32, tag="pt")
                    nc.tensor.transpose(
                        pqt[:, :], qn[:, at * 128:(at + 1) * 128],
                        ident[:nbc, :nbc],
                    )
                    nc.vector.tensor_copy(qT[:, at, :], pqt[:, :])

            def ensure_qproj(up_to_at):
                while qp_state["emitted"] * at_per_chunk < up_to_at:
                    emit_qproj_chunk()

            # -- main loop (software-pipelined).  Slab q's sum-matmul /
            #    exp / transposes / context matmuls are emitted after
            #    slab q+1's projection matmuls: the scores chain runs on
            #    ScalarE+VectorE and gets a full slab period to finish,
            #    so it never stalls the strict-FIFO PE queue. -----------
            state = {}
            pend = {"p": None}

            def emit_epilogue(b):
                w_exp, sume, ctxp = state.pop(b)
                tot = spool.tile([1, 1], F32, tag="tot")
                nc.vector.reduce_sum(tot[:, :], sume[:, :], axis=AXX)
                inv = spool.tile([1, 1], F32, tag="inv")
                nc.vector.reciprocal(inv[:, :], tot[:, :])
                ctx_sb = opool.tile([1, h], F32, tag="ctxsb")
                nc.vector.tensor_scalar_mul(ctx_sb[:, :], ctxp[:, :], inv[:, :])
                nc.sync.dma_start(out=ctx_d.ap()[b:b + 1, :], in_=ctx_sb[:, :])
                attn_sb = opool.tile([1, l], F32, tag="attnsb")
                nc.vector.tensor_scalar_mul(attn_sb[:, :], w_exp[:, :],
                                            inv[:, :])
                nc.sync.dma_start(out=attn_d.ap()[b:b + 1, :], in_=attn_sb[:, :])

            def emit_pending():
                if pend["p"] is None:
                    return
                b, q, acc_r, kN = pend["p"]
                pend["p"] = None
                w_exp, sume, ctxp = state[b]
                ps = pmpool.tile([1, lq], F32, tag="ps")
                nc.tensor.matmul(ps[:, :], ones_r[:, :], acc_r[:, :],
                                 start=True, stop=True)
                nc.scalar.activation(
                    w_exp[:, q * lq:(q + 1) * lq], ps[:, :], EXP,
                    accum_out=sume[:, q:q + 1],
                )
                pt = pmpool.tile([128, lcq], F32, tag="pt")
                for j in range(lcq):
                    nc.tensor.transpose(
                        pt[:, j:j + 1],
                        w_exp[:, q * lq + j * 128:q * lq + (j + 1) * 128],
                        ident[:1, :1],
                    )
                aT = spool.tile([128, lcq], F32R, tag="aT")
                nc.vector.tensor_copy(aT[:, :], pt[:, :])
                for j in range(lcq):
                    for hh2 in range(n_hw):
                        nc.tensor.matmul(
                            ctxp[:, hh2 * hw_chunk:(hh2 + 1) * hw_chunk],
                            aT[:, j:j + 1],
                            kN[:, j, hh2 * hw_chunk:(hh2 + 1) * hw_chunk],
                            start=(q == 0 and j == 0),
                            stop=(q == nlq - 1 and j == lcq - 1),
                        )
                if q == nlq - 1:
                    emit_epilogue(b)

            for b in range(nbc):
                state[b] = (
                    spool.tile([1, l], F32, tag="wexp", name="wexp"),
                    spool.tile([1, nlq], F32, tag="sume", name="sume"),
                    pcpool.tile([1, h], F32, tag="ctx", name="ctxp"),
                )
                for q in range(nlq):
                    # slab DMAs split into pieces: each dma_start lands on
                    # one DMA queue, so splitting multiplies transfer BW
                    kT = kTpool.tile([128, hc, lq], F32R, tag="kT")
                    kT_src = keysT_d.ap()[b].rearrange("(c k) l -> k c l", k=128)[
                        :, :, q * lq:(q + 1) * lq]
                    step = 1 if (b == 0 and q == 0) else 2
                    for hh in range(0, hc, step):
                        nc.sync.dma_start(out=kT[:, hh:hh + step, :],
                                          in_=kT_src[:, hh:hh + step, :])
                    kN = kNpool.tile([128, lcq, h], F32R, tag="kN")
                    kN_src = keysN_d.ap()[b].rearrange("(t k) h -> k t h", k=128)[
                        :, q * lcq:(q + 1) * lcq, :]
                    for t in range(0, lcq, 2):
                        nc.sync.dma_start(out=kN[:, t:t + 2, :],
                                          in_=kN_src[:, t:t + 2, :])
                    en = enpool.tile([128, ac, lq], F32R, tag="en")
                    # k-projection + tanh, a_grp PSUM tiles in flight.
                    # On the very first slab, q-proj chunks are emitted
                    # between a group's matmuls and its tanhs.
                    for g in range(n_ah):
                        pks = []
                        for ai in range(a_grp):
                            at = g * a_grp + ai
                            pk = pkpool.tile([128, lq], F32, tag=f"pk{ai}")
                            pks.append(pk)
                            for hh in range(hc):
                                nc.tensor.matmul(
                                    pk[:, :],
                                    wk[:, hh, at * 128:(at + 1) * 128],
                                    kT[:, hh, :],
                                    start=(hh == 0),
                                    stop=(hh == hc - 1),
                                )
                        if b == 0 and q == 0:
                            ensure_qproj((g + 1) * a_grp)
                        for ai in range(a_grp):
                            at = g * a_grp + ai
                            nc.scalar.activation(
                                en[:, at, :], pks[ai][:, :], TANH,
                                bias=qT[:, at, b:b + 1],
                            )
                    # previous slab's sum/exp/transpose/context work goes
                    # into the PE queue here, behind this slab's matmuls
                    emit_pending()
                    # scores chain for this slab on ScalarE+VectorE only:
                    # m_at = energy[at] * v[at] (per-partition scale),
                    # pairwise adds, last add rounds to fp32r
                    m_prev = None
                    acc = None
                    for at in range(ac):
                        m = opool.tile([128, lq], F32, tag=f"m{at % 2}")
                        nc.scalar.activation(
                            m[:, :], en[:, at, :].bitcast(F32), COPY,
                            scale=v_sb[:, at:at + 1],
                        )
                        if at == 0:
                            m_prev = m
                            continue
                        last = (at == ac - 1)
                        if last:
                            nxt = spool.tile([128, lq], F32R, tag="accr",
                                             name="accr")
                        else:
                            nxt = opool.tile([128, lq], F32,
                                             tag=f"acc{at % 2}",
                                             name=f"acc{at % 2}")
                        nc.vector.tensor_tensor(
                            out=nxt[:, :],
                            in0=(m_prev if at == 1 else acc)[:, :],
                            in1=m[:, :], op=mybir.AluOpType.add)
                        acc = nxt
                    pend["p"] = (b, q, acc, kN)
            emit_pending()

    nc.compile()
    return nc


_NC_CACHE = {}


def _get_nc():
    if "nc" not in _NC_CACHE:
        _NC_CACHE["nc"] = _build()
    return _NC_CACHE["nc"]


def _in_maps(query, keys, W_q, W_k, v):
    query = np.asarray(query, dtype=np.float32)
    keys = np.asarray(keys, dtype=np.float32)
    W_q = np.asarray(W_q, dtype=np.float32)
    W_k = np.asarray(W_k, dtype=np.float32)
    v = np.asarray(v, dtype=np.float32)

    WqT = np.ascontiguousarray(W_q.T)
    WkT = np.ascontiguousarray(W_k.T)

    in_maps = []
    for c in range(N_CORES):
        sl = slice(c * NBC, (c + 1) * NBC)
        in_maps.append({
            "keysT": np.ascontiguousarray(keys[sl].transpose(0, 2, 1)),
            "keysN": np.ascontiguousarray(keys[sl]),
            "queryT": np.ascontiguousarray(query[sl].T),
            "WqT": WqT,
            "WkT": WkT,
            "v": v,
        })
    return in_maps


def _run(in_maps, **kw):
    from concourse.bass_utils import run_bass_kernel_spmd

    nc = _get_nc()
    res = run_bass_kernel_spmd(nc, in_maps, list(range(N_CORES)), **kw)
    context = np.concatenate([r["context"] for r in res.results], axis=0)
    attn = np.concatenate([r["attn"] for r in res.results], axis=0)
    return (context, attn), res


def kernel(query, keys, mask, W_q, W_k, v):
    out, _ = _run(_in_maps(query, keys, W_q, W_k, v))
    return out


# revision 54
# speedup vs baseline: 1.0857x; 1.0152x over previous
"""Additive (Bahdanau) attention TRN2 kernel.

reference:
    q = query @ W_q.T                         # [N,A]
    k = einsum('nlh,ah->nla', keys, W_k)      # [N,L,A]
    energy = tanh(k + q[:,None,:])
    scores = einsum('nla,a->nl', energy, v)   # [N,L]
    attn = softmax(scores, -1)                # mask is all-True: no-op
    context = einsum('nl,nlh->nh', attn, keys)
    return (context, attn)

Sharding: data-parallel over N across 8 cores (4 batches per core);
W_q/W_k/v replicated. Everything runs on one SPMD Bass program.

Per-core layout (per batch b, per l-slab of LQ=512):
  matmul1  k_projT[a, l] += W_kT[h, a].T @ keysT[h, l]   (fp32r, PSUM [128,512] x4)
  ACT      energyT = tanh(k_projT + qT[a])               (bias = per-partition q)
  matmul   scores[1, l]  += v[a].T @ energyT[a, l]       (fp32r)
  ACT      w_exp = exp(scores), accum_out -> slab sum    (no max needed:
           |scores| <= ||v||_1 * 1 ~= 16, exp safe in fp32)
  PE       transpose w_exp chunks -> attnT [l=128, 1]
  matmul   ctx[1, h] += attnT.T @ keysN[l, h]            (fp32r, streaming)
  epilogue context = ctx / sum(exp), attn = w_exp / sum(exp)

keysT (transposed) and keysN (natural) copies are prepared host-side so
every DMA is wide and contiguous; the contraction dim always sits on SBUF
partitions.
"""

import numpy as np

N, L, H, A = 32, 2048, 1024, 1024
N_CORES = 8
NBC = N // N_CORES  # batches per core


def _build(nbc=NBC, l=L, h=H, a=A, lq=512, a_grp=4):
    import concourse.mybir as mybir
    from concourse import bacc
    from concourse.tile import TileContext

    F32 = mybir.dt.float32
    F32R = mybir.dt.float32r
    TANH = mybir.ActivationFunctionType.Tanh
    EXP = mybir.ActivationFunctionType.Exp
    COPY = mybir.ActivationFunctionType.Copy
    AXX = mybir.AxisListType.X

    hc = h // 128      # h-chunks (contraction for matmul1)
    ac = a // 128      # a-tiles
    nlq = l // lq      # l-slabs per batch
    lcq = lq // 128    # 128-chunks per l-slab
    n_ah = ac // a_grp # a-groups (psum tiles in flight)
    hw_chunk = 512 if h % 512 == 0 else h  # ctx matmul N-chunk (<=1 PSUM bank)
    n_hw = h // hw_chunk

    nc = bacc.Bacc("TRN2", target_bir_lowering=False)

    keysT_d = nc.dram_tensor("keysT", [nbc, h, l], F32R, kind="ExternalInput")
    keysN_d = nc.dram_tensor("keysN", [nbc, l, h], F32R, kind="ExternalInput")
    qryT_d = nc.dram_tensor("queryT", [h, nbc], F32R, kind="ExternalInput")
    WqT_d = nc.dram_tensor("WqT", [h, a], F32R, kind="ExternalInput")
    WkT_d = nc.dram_tensor("WkT", [h, a], F32R, kind="ExternalInput")
    v_d = nc.dram_tensor("v", [a], F32, kind="ExternalInput")
    ctx_d = nc.dram_tensor("context", [nbc, h], F32, kind="ExternalOutput")
    attn_d = nc.dram_tensor("attn", [nbc, l], F32, kind="ExternalOutput")

    with TileContext(nc) as tc:
        with (
            tc.tile_pool(name="const", bufs=1) as cpool,
            tc.tile_pool(name="kT", bufs=2) as kTpool,
            tc.tile_pool(name="wq", bufs=1) as wqpool,
            tc.tile_pool(name="kN", bufs=2) as kNpool,
            tc.tile_pool(name="en", bufs=2) as enpool,
            tc.tile_pool(name="small", bufs=2) as spool,
            tc.tile_pool(name="outs", bufs=1) as opool,
            tc.tile_pool(name="pk", bufs=1, space="PSUM") as pkpool,
            tc.tile_pool(name="pmisc", bufs=1, space="PSUM") as pmpool,
            tc.tile_pool(name="pctx", bufs=1, space="PSUM") as pcpool,
        ):
            # -- resident constants.  WkT pieces go first on the Sync
            #    issue queue (the first matmuls need them); query/v ride
            #    the GpSimd queue in parallel. -------------------------
            from concourse.masks import make_identity

            wq_v = WqT_d.ap().rearrange("(c k) a -> k c a", k=128)
            wk = cpool.tile([128, hc, a], F32R)
            wk_v = WkT_d.ap().rearrange("(c k) a -> k c a", k=128)
            for hh in range(0, hc, 2):
                nc.sync.dma_start(out=wk[:, hh:hh + 2, :],
                                  in_=wk_v[:, hh:hh + 2, :])
            qry = cpool.tile([128, hc, nbc], F32R)
            nc.gpsimd.dma_start(out=qry[:, :, :],
                                in_=qryT_d.ap().rearrange("(c k) n -> k c n",
                                                          k=128))
            v_sb = cpool.tile([128, ac], F32)
            nc.gpsimd.dma_start(out=v_sb[:, :],
                                in_=v_d.ap().rearrange("(c k) -> k c", k=128))
            ident = cpool.tile([32, 32], F32)
            make_identity(nc, ident[:, :])
            ones32 = cpool.tile([128, 1], F32)
            nc.gpsimd.memset(ones32[:, :], 1.0)
            ones_r = cpool.tile([128, 1], F32R)
            nc.vector.tensor_copy(ones_r[:, :], ones32[:, :])
            v_r = cpool.tile([128, ac], F32R)
            nc.vector.tensor_copy(v_r[:, :], v_sb[:, :])

            # -- q projection: q[n, a] = sum_h queryT[h, n] * WqT[h, a],
            #    then PE-transpose to qT[a-part, n].  Emitted lazily,
            #    interleaved with the first keys slab's matmuls so the PE
            #    starts on mm1 as soon as WkT/keysT land. ---------------
            qn = opool.tile([nbc, a], F32, tag="qn")
            qT = cpool.tile([128, ac, nbc], F32)
            at_per_chunk = hw_chunk // 128
            qp_state = {"emitted": 0}

            def emit_qproj_chunk():
                hh2 = qp_state["emitted"]
                qp_state["emitted"] += 1
                wqc = wqpool.tile([128, hc, hw_chunk], F32R, tag="wq")
                for hh in range(hc):
                    nc.sync.dma_start(
                        out=wqc[:, hh, :],
                        in_=wq_v[:, hh, hh2 * hw_chunk:(hh2 + 1) * hw_chunk])
                pq = pmpool.tile([128, hw_chunk], F32, tag="ps")
                for hh in range(hc):
                    nc.tensor.matmul(
                        pq[:nbc, :],
                        qry[:, hh, :],
                        wqc[:, hh, :],
                        start=(hh == 0),
                        stop=(hh == hc - 1),
                    )
                nc.vector.tensor_copy(
                    qn[:, hh2 * hw_chunk:(hh2 + 1) * hw_chunk], pq[:nbc, :])
                for at in range(hh2 * at_per_chunk, (hh2 + 1) * at_per_chunk):
                    pqt = pmpool.tile([128, nbc], F32, tag="pt")
                    nc.tensor.transpose(
                        pqt[:, :], qn[:, at * 128:(at + 1) * 128],
                        ident[:nbc, :nbc],
                    )
                    nc.vector.tensor_copy(qT[:, at, :], pqt[:, :])

            def ensure_qproj(up_to_at):
                while qp_state["emitted"] * at_per_chunk < up_to_at:
                    emit_qproj_chunk()

            # -- main loop (software-pipelined).  Slab q's sum-matmul /
            #    exp / transposes / context matmuls are emitted after
            #    slab q+1's projection matmuls: the scores chain runs on
            #    ScalarE+VectorE and gets a full slab period to finish,
            #    so it never stalls the strict-FIFO PE queue. -----------
            state = {}
            pend = {"p": None}

            def emit_epilogue(b):
                w_exp, sume, ctxp = state.pop(b)
                tot = spool.tile([1, 1], F32, tag="tot")
                nc.vector.reduce_sum(tot[:, :], sume[:, :], axis=AXX)
                inv = spool.tile([1, 1], F32, tag="inv")
                nc.vector.reciprocal(inv[:, :], tot[:, :])
                ctx_sb = opool.tile([1, h], F32, tag="ctxsb")
                nc.vector.tensor_scalar_mul(ctx_sb[:, :], ctxp[:, :], inv[:, :])
                nc.sync.dma_start(out=ctx_d.ap()[b:b + 1, :], in_=ctx_sb[:, :])
                attn_sb = opool.tile([1, l], F32, tag="attnsb")
                nc.vector.tensor_scalar_mul(attn_sb[:, :], w_exp[:, :],
                                            inv[:, :])
                nc.sync.dma_start(out=attn_d.ap()[b:b + 1, :], in_=attn_sb[:, :])

            def emit_pending():
                if pend["p"] is None:
                    return
                b, q, src_kind, src, kN = pend["p"]
                pend["p"] = None
                w_exp, sume, ctxp = state[b]
                if src_kind == "chain":
                    ps = pmpool.tile([1, lq], F32, tag="ps")
                    nc.tensor.matmul(ps[:, :], ones_r[:, :], src[:, :],
                                     start=True, stop=True)
                else:
                    ps = src
                nc.scalar.activation(
                    w_exp[:, q * lq:(q + 1) * lq], ps[:, :], EXP,
                    accum_out=sume[:, q:q + 1],
                )
                pt = pmpool.tile([128, lcq], F32, tag="pt")
                for j in range(lcq):
                    nc.tensor.transpose(
                        pt[:, j:j + 1],
                        w_exp[:, q * lq + j * 128:q * lq + (j + 1) * 128],
                        ident[:1, :1],
                    )
                aT = spool.tile([128, lcq], F32R, tag="aT")
                nc.vector.tensor_copy(aT[:, :], pt[:, :])
                for j in range(lcq):
                    for hh2 in range(n_hw):
                        nc.tensor.matmul(
                            ctxp[:, hh2 * hw_chunk:(hh2 + 1) * hw_chunk],
                            aT[:, j:j + 1],
                            kN[:, j, hh2 * hw_chunk:(hh2 + 1) * hw_chunk],
                            start=(q == 0 and j == 0),
                            stop=(q == nlq - 1 and j == lcq - 1),
                        )
                if q == nlq - 1:
                    emit_epilogue(b)

            for b in range(nbc):
                state[b] = (
                    spool.tile([1, l], F32, tag="wexp", name="wexp"),
                    spool.tile([1, nlq], F32, tag="sume", name="sume"),
                    pcpool.tile([1, h], F32, tag="ctx", name="ctxp"),
                )
                for q in range(nlq):
                    # slab DMAs split into pieces: each dma_start lands on
                    # one DMA queue, so splitting multiplies transfer BW
                    kT = kTpool.tile([128, hc, lq], F32R, tag="kT")
                    kT_src = keysT_d.ap()[b].rearrange("(c k) l -> k c l", k=128)[
                        :, :, q * lq:(q + 1) * lq]
                    step = 1 if (b == 0 and q <= 1) else 2
                    for hh in range(0, hc, step):
                        nc.sync.dma_start(out=kT[:, hh:hh + step, :],
                                          in_=kT_src[:, hh:hh + step, :])
                    kN = kNpool.tile([128, lcq, h], F32R, tag="kN")
                    kN_src = keysN_d.ap()[b].rearrange("(t k) h -> k t h", k=128)[
                        :, q * lcq:(q + 1) * lcq, :]
                    for t in range(0, lcq, 2):
                        nc.sync.dma_start(out=kN[:, t:t + 2, :],
                                          in_=kN_src[:, t:t + 2, :])
                    en = enpool.tile([128, ac, lq], F32R, tag="en")
                    # k-projection + tanh, a_grp PSUM tiles in flight.
                    # On the very first slab, q-proj chunks are emitted
                    # between a group's matmuls and its tanhs.
                    for g in range(n_ah):
                        pks = []
                        for ai in range(a_grp):
                            at = g * a_grp + ai
                            pk = pkpool.tile([128, lq], F32, tag=f"pk{ai}")
                            pks.append(pk)
                            for hh in range(hc):
                                nc.tensor.matmul(
                                    pk[:, :],
                                    wk[:, hh, at * 128:(at + 1) * 128],
                                    kT[:, hh, :],
                                    start=(hh == 0),
                                    stop=(hh == hc - 1),
                                )
                        if b == 0 and q == 0:
                            ensure_qproj((g + 1) * a_grp)
                        for ai in range(a_grp):
                            at = g * a_grp + ai
                            nc.scalar.activation(
                                en[:, at, :], pks[ai][:, :], TANH,
                                bias=qT[:, at, b:b + 1],
                            )
                    # previous slab's sum/exp/transpose/context work goes
                    # into the PE queue here, behind this slab's matmuls
                    emit_pending()
                    if b == nbc - 1 and q == nlq - 1:
                        # final slab: no next-slab matmuls to hide the
                        # ACT/DVE chain behind — PE scores drain faster
                        ps = pmpool.tile([1, lq], F32, tag="ps")
                        for at in range(ac):
                            nc.tensor.matmul(
                                ps[:, :], v_r[:, at:at + 1], en[:, at, :],
                                start=(at == 0), stop=(at == ac - 1))
                        pend["p"] = (b, q, "psum", ps, kN)
                        emit_pending()
                        continue
                    # scores chain for this slab on ScalarE+VectorE only:
                    # m_at = energy[at] * v[at] (per-partition scale),
                    # pairwise adds, last add rounds to fp32r
                    m_prev = None
                    acc = None
                    for at in range(ac):
                        m = opool.tile([128, lq], F32, tag=f"m{at % 2}")
                        nc.scalar.activation(
                            m[:, :], en[:, at, :].bitcast(F32), COPY,
                            scale=v_sb[:, at:at + 1],
                        )
                        if at == 0:
                            m_prev = m
                            continue
                        last = (at == ac - 1)
                        if last:
                            nxt = spool.tile([128, lq], F32R, tag="accr",
                                             name="accr")
                        else:
                            nxt = opool.tile([128, lq], F32,
                                             tag=f"acc{at % 2}",
                                             name=f"acc{at % 2}")
                        nc.vector.tensor_tensor(
                            out=nxt[:, :],
                            in0=(m_prev if at == 1 else acc)[:, :],
                            in1=m[:, :], op=mybir.AluOpType.add)
                        acc = nxt
                    pend["p"] = (b, q, "chain", acc, kN)
            emit_pending()

    nc.compile()
    return nc


_NC_CACHE = {}


def _get_nc():
    if "nc" not in _NC_CACHE:
        _NC_CACHE["nc"] = _build()
    return _NC_CACHE["nc"]


def _in_maps(query, keys, W_q, W_k, v):
    query = np.asarray(query, dtype=np.float32)
    keys = np.asarray(keys, dtype=np.float32)
    W_q = np.asarray(W_q, dtype=np.float32)
    W_k = np.asarray(W_k, dtype=np.float32)
    v = np.asarray(v, dtype=np.float32)

    WqT = np.ascontiguousarray(W_q.T)
    WkT = np.ascontiguousarray(W_k.T)

    in_maps = []
    for c in range(N_CORES):
        sl = slice(c * NBC, (c + 1) * NBC)
        in_maps.append({
            "keysT": np.ascontiguousarray(keys[sl].transpose(0, 2, 1)),
            "keysN": np.ascontiguousarray(keys[sl]),
            "queryT": np.ascontiguousarray(query[sl].T),
            "WqT": WqT,
            "WkT": WkT,
            "v": v,
        })
    return in_maps


def _run(in_maps, **kw):
    from concourse.bass_utils import run_bass_kernel_spmd

    nc = _get_nc()
    res = run_bass_kernel_spmd(nc, in_maps, list(range(N_CORES)), **kw)
    context = np.concatenate([r["context"] for r in res.results], axis=0)
    attn = np.concatenate([r["attn"] for r in res.results], axis=0)
    return (context, attn), res


def kernel(query, keys, mask, W_q, W_k, v):
    out, _ = _run(_in_maps(query, keys, W_q, W_k, v))
    return out


# revision 58
# speedup vs baseline: 1.2697x; 1.1694x over previous
"""Additive (Bahdanau) attention TRN2 kernel.

reference:
    q = query @ W_q.T                         # [N,A]
    k = einsum('nlh,ah->nla', keys, W_k)      # [N,L,A]
    energy = tanh(k + q[:,None,:])
    scores = einsum('nla,a->nl', energy, v)   # [N,L]
    attn = softmax(scores, -1)                # mask is all-True: no-op
    context = einsum('nl,nlh->nh', attn, keys)
    return (context, attn)

Sharding: data-parallel over N across 8 cores (4 batches per core);
W_q/W_k/v replicated. Everything runs on one SPMD Bass program.

Per-core layout (per batch b, per l-slab of LQ=512):
  matmul1  k_projT[a, l] += W_kT[h, a].T @ keysT[h, l]   (fp32r, PSUM [128,512] x4)
  ACT      energyT = tanh(k_projT + qT[a])               (bias = per-partition q)
  matmul   scores[1, l]  += v[a].T @ energyT[a, l]       (fp32r)
  ACT      w_exp = exp(scores), accum_out -> slab sum    (no max needed:
           |scores| <= ||v||_1 * 1 ~= 16, exp safe in fp32)
  PE       transpose w_exp chunks -> attnT [l=128, 1]
  matmul   ctx[1, h] += attnT.T @ keysN[l, h]            (fp32r, streaming)
  epilogue context = ctx / sum(exp), attn = w_exp / sum(exp)

keysT (transposed) and keysN (natural) copies are prepared host-side so
every DMA is wide and contiguous; the contraction dim always sits on SBUF
partitions.
"""

import numpy as np

N, L, H, A = 32, 2048, 1024, 1024
N_CORES = 8
NBC = N // N_CORES  # batches per core


def _build(nbc=NBC, l=L, h=H, a=A, lq=512, a_grp=4):
    import concourse.mybir as mybir
    from concourse import bacc
    from concourse.tile import TileContext

    F32 = mybir.dt.float32
    F32R = mybir.dt.float32r
    TANH = mybir.ActivationFunctionType.Tanh
    EXP = mybir.ActivationFunctionType.Exp
    COPY = mybir.ActivationFunctionType.Copy
    AXX = mybir.AxisListType.X

    hc = h // 128      # h-chunks (contraction for matmul1)
    ac = a // 128      # a-tiles
    nlq = l // lq      # l-slabs per batch
    lcq = lq // 128    # 128-chunks per l-slab
    n_ah = ac // a_grp # a-groups (psum tiles in flight)
    hw_chunk = 512 if h % 512 == 0 else h  # ctx matmul N-chunk (<=1 PSUM bank)
    n_hw = h // hw_chunk

    nc = bacc.Bacc("TRN2", target_bir_lowering=False)

    keysT_d = nc.dram_tensor("keysT", [nbc, h, l], F32R, kind="ExternalInput")
    keysN_d = nc.dram_tensor("keysN", [nbc, l, h], F32R, kind="ExternalInput")
    qryT_d = nc.dram_tensor("queryT", [h, nbc], F32R, kind="ExternalInput")
    WqT_d = nc.dram_tensor("WqT", [h, a], F32R, kind="ExternalInput")
    WkT_d = nc.dram_tensor("WkT", [h, a], F32R, kind="ExternalInput")
    v_d = nc.dram_tensor("v", [a], F32, kind="ExternalInput")
    ctx_d = nc.dram_tensor("context", [nbc, h], F32, kind="ExternalOutput")
    attn_d = nc.dram_tensor("attn", [nbc, l], F32, kind="ExternalOutput")

    with TileContext(nc) as tc:
        with (
            tc.tile_pool(name="const", bufs=1) as cpool,
            tc.tile_pool(name="kT", bufs=2) as kTpool,
            tc.tile_pool(name="wq", bufs=1) as wqpool,
            tc.tile_pool(name="kN", bufs=2) as kNpool,
            tc.tile_pool(name="en", bufs=2) as enpool,
            tc.tile_pool(name="small", bufs=2) as spool,
            tc.tile_pool(name="outs", bufs=1) as opool,
            tc.tile_pool(name="pk", bufs=1, space="PSUM") as pkpool,
            tc.tile_pool(name="pmisc", bufs=1, space="PSUM") as pmpool,
            tc.tile_pool(name="pctx", bufs=1, space="PSUM") as pcpool,
        ):
            # -- resident constants.  WkT pieces go first on the Sync
            #    issue queue (the first matmuls need them); query/v ride
            #    the GpSimd queue in parallel. -------------------------
            from concourse.masks import make_identity

            wq_v = WqT_d.ap().rearrange("(c k) a -> k c a", k=128)
            wk = cpool.tile([128, hc, a], F32R)
            wk_v = WkT_d.ap().rearrange("(c k) a -> k c a", k=128)
            for hh in range(0, hc, 2):
                nc.sync.dma_start(out=wk[:, hh:hh + 2, :],
                                  in_=wk_v[:, hh:hh + 2, :])
            qry = cpool.tile([128, hc, nbc], F32R)
            nc.gpsimd.dma_start(out=qry[:, :, :],
                                in_=qryT_d.ap().rearrange("(c k) n -> k c n",
                                                          k=128))
            v_sb = cpool.tile([128, ac], F32)
            nc.gpsimd.dma_start(out=v_sb[:, :],
                                in_=v_d.ap().rearrange("(c k) -> k c", k=128))
            ident = cpool.tile([32, 32], F32)
            make_identity(nc, ident[:, :])
            ones32 = cpool.tile([128, 1], F32)
            nc.gpsimd.memset(ones32[:, :], 1.0)
            ones_r = cpool.tile([128, 1], F32R)
            nc.vector.tensor_copy(ones_r[:, :], ones32[:, :])
            v_r = cpool.tile([128, ac], F32R)
            nc.vector.tensor_copy(v_r[:, :], v_sb[:, :])

            # -- q projection: q[n, a] = sum_h queryT[h, n] * WqT[h, a],
            #    then PE-transpose to qT[a-part, n].  Emitted lazily,
            #    interleaved with the first keys slab's matmuls so the PE
            #    starts on mm1 as soon as WkT/keysT land. ---------------
            qn = opool.tile([nbc, a], F32, tag="qn")
            qT = cpool.tile([128, ac, nbc], F32)
            at_per_chunk = hw_chunk // 128
            qp_state = {"emitted": 0}

            def emit_qproj_chunk():
                hh2 = qp_state["emitted"]
                qp_state["emitted"] += 1
                wqc = wqpool.tile([128, hc, hw_chunk], F32R, tag="wq")
                for hh in range(hc):
                    nc.sync.dma_start(
                        out=wqc[:, hh, :],
                        in_=wq_v[:, hh, hh2 * hw_chunk:(hh2 + 1) * hw_chunk])
                pq = pmpool.tile([128, hw_chunk], F32, tag="ps")
                for hh in range(hc):
                    nc.tensor.matmul(
                        pq[:nbc, :],
                        qry[:, hh, :],
                        wqc[:, hh, :],
                        start=(hh == 0),
                        stop=(hh == hc - 1),
                    )
                nc.vector.tensor_copy(
                    qn[:, hh2 * hw_chunk:(hh2 + 1) * hw_chunk], pq[:nbc, :])
                for at in range(hh2 * at_per_chunk, (hh2 + 1) * at_per_chunk):
                    pqt = pmpool.tile([128, nbc], F32, tag="pt")
                    nc.tensor.transpose(
                        pqt[:, :], qn[:, at * 128:(at + 1) * 128],
                        ident[:nbc, :nbc],
                    )
                    nc.vector.tensor_copy(qT[:, at, :], pqt[:, :])

            def ensure_qproj(up_to_at):
                while qp_state["emitted"] * at_per_chunk < up_to_at:
                    emit_qproj_chunk()

            # -- main loop (software-pipelined).  Slab q's sum-matmul /
            #    exp / transposes / context matmuls are emitted after
            #    slab q+1's projection matmuls: the scores chain runs on
            #    ScalarE+VectorE and gets a full slab period to finish,
            #    so it never stalls the strict-FIFO PE queue. -----------
            state = {}
            pend = {"p": None}

            def emit_epilogue(b):
                w_exp, sume, ctxp = state.pop(b)
                tot = spool.tile([1, 1], F32, tag="tot")
                nc.vector.reduce_sum(tot[:, :], sume[:, :], axis=AXX)
                inv = spool.tile([1, 1], F32, tag="inv")
                nc.vector.reciprocal(inv[:, :], tot[:, :])
                ctx_sb = opool.tile([1, h], F32, tag="ctxsb")
                nc.vector.tensor_scalar_mul(ctx_sb[:, :], ctxp[:, :], inv[:, :])
                nc.sync.dma_start(out=ctx_d.ap()[b:b + 1, :], in_=ctx_sb[:, :])
                attn_sb = opool.tile([1, l], F32, tag="attnsb")
                nc.vector.tensor_scalar_mul(attn_sb[:, :], w_exp[:, :],
                                            inv[:, :])
                nc.sync.dma_start(out=attn_d.ap()[b:b + 1, :], in_=attn_sb[:, :])

            def emit_pending_sum():
                p = pend["p"]
                if p is None or p.get("sum_done"):
                    return
                p["sum_done"] = True
                b, q, src_kind, src = p["b"], p["q"], p["kind"], p["src"]
                w_exp, sume, ctxp = state[b]
                if src_kind == "chain":
                    ps = pmpool.tile([1, lq], F32, tag="ps")
                    nc.tensor.matmul(ps[:, :], ones_r[:, :], src[:, :],
                                     start=True, stop=True)
                else:
                    ps = src
                nc.scalar.activation(
                    w_exp[:, q * lq:(q + 1) * lq], ps[:, :], EXP,
                    accum_out=sume[:, q:q + 1],
                )

            def emit_pending():
                if pend["p"] is None:
                    return
                emit_pending_sum()
                b, q, kN = pend["p"]["b"], pend["p"]["q"], pend["p"]["kN"]
                pend["p"] = None
                w_exp, sume, ctxp = state[b]
                pt = pmpool.tile([128, lcq], F32, tag="pt")
                for j in range(lcq):
                    nc.tensor.transpose(
                        pt[:, j:j + 1],
                        w_exp[:, q * lq + j * 128:q * lq + (j + 1) * 128],
                        ident[:1, :1],
                    )
                aT = spool.tile([128, lcq], F32R, tag="aT")
                nc.vector.tensor_copy(aT[:, :], pt[:, :])
                for j in range(lcq):
                    for hh2 in range(n_hw):
                        nc.tensor.matmul(
                            ctxp[:, hh2 * hw_chunk:(hh2 + 1) * hw_chunk],
                            aT[:, j:j + 1],
                            kN[:, j, hh2 * hw_chunk:(hh2 + 1) * hw_chunk],
                            start=(q == 0 and j == 0),
                            stop=(q == nlq - 1 and j == lcq - 1),
                        )
                if q == nlq - 1:
                    emit_epilogue(b)

            for b in range(nbc):
                state[b] = (
                    spool.tile([1, l], F32, tag="wexp", name="wexp"),
                    spool.tile([1, nlq], F32, tag="sume", name="sume"),
                    pcpool.tile([1, h], F32, tag="ctx", name="ctxp"),
                )
                for q in range(nlq):
                    # slab DMAs split into pieces: each dma_start lands on
                    # one DMA queue, so splitting multiplies transfer BW
                    kT = kTpool.tile([128, hc, lq], F32R, tag="kT")
                    kT_src = keysT_d.ap()[b].rearrange("(c k) l -> k c l", k=128)[
                        :, :, q * lq:(q + 1) * lq]
                    step = 1 if (b == 0 and q <= 1) else 2
                    for hh in range(0, hc, step):
                        nc.sync.dma_start(out=kT[:, hh:hh + step, :],
                                          in_=kT_src[:, hh:hh + step, :])
                    kN = kNpool.tile([128, lcq, h], F32R, tag="kN")
                    kN_src = keysN_d.ap()[b].rearrange("(t k) h -> k t h", k=128)[
                        :, q * lcq:(q + 1) * lcq, :]
                    for t in range(0, lcq, 2):
                        nc.sync.dma_start(out=kN[:, t:t + 2, :],
                                          in_=kN_src[:, t:t + 2, :])
                    en = enpool.tile([128, ac, lq], F32R, tag="en")
                    # k-projection + tanh, a_grp PSUM tiles in flight.
                    # On the very first slab, q-proj chunks are emitted
                    # between a group's matmuls and its tanhs.
                    for g in range(n_ah):
                        pks = []
                        for ai in range(a_grp):
                            at = g * a_grp + ai
                            pk = pkpool.tile([128, lq], F32, tag=f"pk{ai}")
                            pks.append(pk)
                            for hh in range(hc):
                                nc.tensor.matmul(
                                    pk[:, :],
                                    wk[:, hh, at * 128:(at + 1) * 128],
                                    kT[:, hh, :],
                                    start=(hh == 0),
                                    stop=(hh == hc - 1),
                                )
                        if b == 0 and q == 0:
                            ensure_qproj((g + 1) * a_grp)
                        for ai in range(a_grp):
                            at = g * a_grp + ai
                            nc.scalar.activation(
                                en[:, at, :], pks[ai][:, :], TANH,
                                bias=qT[:, at, b:b + 1],
                            )
                        if g == 0:
                            # previous slab's scores-sum + exp slot in
                            # behind the first matmul group; exp then has
                            # a full group's time before its transposes
                            emit_pending_sum()
                    # previous slab's sum/exp/transpose/context work goes
                    # into the PE queue here, behind this slab's matmuls
                    emit_pending()
                    if b == nbc - 1 and q == nlq - 1:
                        # final slab: no next-slab matmuls to hide the
                        # ACT/DVE chain behind — PE scores drain faster
                        ps = pmpool.tile([1, lq], F32, tag="ps")
                        for at in range(ac):
                            nc.tensor.matmul(
                                ps[:, :], v_r[:, at:at + 1], en[:, at, :],
                                start=(at == 0), stop=(at == ac - 1))
                        pend["p"] = {"b": b, "q": q, "kind": "psum",
                                     "src": ps, "kN": kN}
                        emit_pending()
                        continue
                    # scores chain for this slab on ScalarE+VectorE only:
                    # m_at = energy[at] * v[at] (per-partition scale),
                    # pairwise adds, last add rounds to fp32r
                    m_prev = None
                    acc = None
                    for at in range(ac):
                        m = opool.tile([128, lq], F32, tag=f"m{at % 2}")
                        nc.scalar.activation(
                            m[:, :], en[:, at, :].bitcast(F32), COPY,
                            scale=v_sb[:, at:at + 1],
                        )
                        if at == 0:
                            m_prev = m
                            continue
                        last = (at == ac - 1)
                        if last:
                            nxt = spool.tile([128, lq], F32R, tag="accr",
                                             name="accr")
                        else:
                            nxt = opool.tile([128, lq], F32,
                                             tag=f"acc{at % 2}",
                                             name=f"acc{at % 2}")
                        nc.vector.tensor_tensor(
                            out=nxt[:, :],
                            in0=(m_prev if at == 1 else acc)[:, :],
                            in1=m[:, :], op=mybir.AluOpType.add)
                        acc = nxt
                    pend["p"] = {"b": b, "q": q, "kind": "chain",
                                 "src": acc, "kN": kN}
            emit_pending()

    nc.compile()
    return nc


_NC_CACHE = {}


def _get_nc():
    if "nc" not in _NC_CACHE:
        _NC_CACHE["nc"] = _build()
    return _NC_CACHE["nc"]


def _in_maps(query, keys, W_q, W_k, v):
    query = np.asarray(query, dtype=np.float32)
    keys = np.asarray(keys, dtype=np.float32)
    W_q = np.asarray(W_q, dtype=np.float32)
    W_k = np.asarray(W_k, dtype=np.float32)
    v = np.asarray(v, dtype=np.float32)

    WqT = np.ascontiguousarray(W_q.T)
    WkT = np.ascontiguousarray(W_k.T)

    in_maps = []
    for c in range(N_CORES):
        sl = slice(c * NBC, (c + 1) * NBC)
        in_maps.append({
            "keysT": np.ascontiguousarray(keys[sl].transpose(0, 2, 1)),
            "keysN": np.ascontiguousarray(keys[sl]),
            "queryT": np.ascontiguousarray(query[sl].T),
            "WqT": WqT,
            "WkT": WkT,
            "v": v,
        })
    return in_maps


def _run(in_maps, **kw):
    from concourse.bass_utils import run_bass_kernel_spmd

    nc = _get_nc()
    res = run_bass_kernel_spmd(nc, in_maps, list(range(N_CORES)), **kw)
    context = np.concatenate([r["context"] for r in res.results], axis=0)
    attn = np.concatenate([r["attn"] for r in res.results], axis=0)
    return (context, attn), res


def kernel(query, keys, mask, W_q, W_k, v):
    out, _ = _run(_in_maps(query, keys, W_q, W_k, v))
    return out
